# revision 38
# baseline (speedup 1.0000x reference)
"""Distributional (Gaussian-KL) attention on 8 TRN2 NeuronCores.

Math: for each head, the KL-based score decomposes as
    kl[q,k] = sum_d (Qm-Km)^2/(2Kv) + 0.5*(Qv/Kv - log(Qv/Kv) - 1)
            = Fq[q] . Fk[k] + r[k] + c[q]
with  Fq = [Qm^2+Qv ; -Qm],  Fk = [1/(2Kv) ; Km/Kv],
      r  = 0.5*sum_d (Km^2/Kv + log Kv),
and c[q] only shifts softmax logits per-row (drops out of softmax over k).

Sharding: head-parallel front (core c owns heads {2c,2c+1} == feature
columns [128c,128c+128) of every Q/K/V projection).  Output stage is
ROW-split: core j owns out rows [128*rblk, +128) x cols [512*cblk, +512)
of ONE half (j<4: mu, j>=4: var), rblk=(j%4)//2, cblk=j%2.  One 256KB
AllToAll (chunk j = our O_half[:, rows_j], 32KB) delivers each core the
full 1024-feature contraction for its rows; out-proj runs at full PE
utilization (lhsT = [128 feats, 128 rows], rhs = 512-col wo block).

Scores per (head, q-tile): one 128-contraction matmul of packed
features fqcat=[Fq1;-Qm] x fkcat=[1/2Kv;Km/Kv], plus one matmul adding
r via an exact bf16 hi+lo split of t_s = Km^2/Kv + lnKv against an
all-0.5 lhsT (hi rows sum exactly in fp32 PSUM, lo carries residuals).

Engine budget: input DMAs fan out over 5 queues (tensor/scalar/gpsimd/
vector/sync) so projections never starve; the 1MB wo block + biases
load late (wo is only needed ~40us in).  ACT does tables + all
PSUM->SBUF copies + softmax normalize (copy/identity are in every ACT
table set, so no table thrash); DVE keeps the feature chain,
reciprocals, and squares.  Output bias is a rank-1 matmul.
"""

import numpy as np

import concourse.bass as bass
import concourse.mybir as mybir
import concourse.tile as tile
from concourse import bacc
from concourse.masks import make_identity
from concourse.bass_utils import run_bass_kernel_spmd

F32 = mybir.dt.float32
BF16 = mybir.dt.bfloat16
AF = mybir.ActivationFunctionType
ALU = mybir.AluOpType

H, B, L, D = 16, 1, 256, 1024
Dh = D // H          # 64
NCORES = 8
CB = D // NCORES     # 128 feature columns per core (2 heads)
P = 128
LT = L // P          # 2 row tiles of the sequence
KT = D // P          # 8 contraction tiles
NW = 6               # projection weight slots: kv qv vv km qm vm
OC = 512             # out-proj column block per core

TRACE = False
TRACE_KWARGS = {}
LAST_RESULT = None

_prog_cache = {}


def ts(i, size):
    return slice(i * size, (i + 1) * size)


def build_program():
    nc = bacc.Bacc("TRN2", target_bir_lowering=False, debug=False,
                   num_devices=NCORES)

    # xcat[p, s, kt, l] = x_s[l, kt*128+p]; s: 0=var, 1=mu
    xcat_d = nc.dram_tensor("xcat", [P, 2, KT, L], BF16, kind="ExternalInput")
    # wcat[p, w, kt, m] = w[kt*128+p, 128c+m]; slots kv qv vv km qm vm
    wcat_d = nc.dram_tensor("wcat", [P, NW, KT, CB], BF16,
                            kind="ExternalInput")
    # wocat[p, kt, m] = wo_half[kt*128+p, 512*cblk+m]
    wocat_d = nc.dram_tensor("wocat", [P, KT, OC], BF16,
                             kind="ExternalInput")
    b_d = nc.dram_tensor("biases", [CB, NW], F32, kind="ExternalInput")
    bo_d = nc.dram_tensor("b_out", [1, OC], F32, kind="ExternalInput")
    # out_a: identity epilogue (mu), out_b: softplus epilogue (var)
    out_a_d = nc.dram_tensor("out_a", [P, OC], F32, kind="ExternalOutput")
    out_b_d = nc.dram_tensor("out_b", [P, OC], F32, kind="ExternalOutput")

    with tile.TileContext(nc) as tc:
        _build(nc, tc, xcat_d, wcat_d, wocat_d, b_d, bo_d, out_a_d, out_b_d)
    nc.compile()
    return nc


def _build(nc, tc, xcat_d, wcat_d, wocat_d, b_d, bo_d, out_a_d, out_b_d):
    from contextlib import ExitStack
    ctx = ExitStack()
    with ctx:
        const = ctx.enter_context(tc.tile_pool(name="const", bufs=1))
        persist = ctx.enter_context(tc.tile_pool(name="persist", bufs=1))
        stage = ctx.enter_context(tc.tile_pool(name="stage", bufs=1))
        feat = ctx.enter_context(tc.tile_pool(name="feat", bufs=1))
        attnp = ctx.enter_context(tc.tile_pool(name="attnp", bufs=2))
        ps_proj = ctx.enter_context(
            tc.tile_pool(name="ps_proj", bufs=2, space="PSUM"))
        ps_tr = ctx.enter_context(
            tc.tile_pool(name="ps_tr", bufs=2, space="PSUM"))
        ps_s = ctx.enter_context(
            tc.tile_pool(name="ps_s", bufs=2, space="PSUM"))
        ps_pv = ctx.enter_context(
            tc.tile_pool(name="ps_pv", bufs=1, space="PSUM"))
        dram = ctx.enter_context(tc.tile_pool(name="dram", bufs=1,
                                              space="DRAM"))

        # ------- input DMAs: 5 issue queues, critical operands first ----
        x_sb = persist.tile([P, 2, KT, L], BF16, tag="x_sb", name="x_sb")
        w_sb = persist.tile([P, NW, KT, CB], BF16, tag="w_sb", name="w_sb")
        wo_sb = persist.tile([P, KT, OC], BF16, tag="wo_sb", name="wo_sb")
        QH = KT // 2
        # w slots in consumption order: kv qv vv km qm vm.  Early window
        # runs ~55GB/s per queue, so x rides two queues and no queue
        # carries more than ~384KB of the first-needed bytes.
        nc.sync.dma_start(x_sb[:, 0, 0:2], xcat_d.ap()[:, 0, 0:2])
        nc.scalar.dma_start(w_sb[:, 0, 0:QH], wcat_d.ap()[:, 0, 0:QH])
        bcat = const.tile([CB, NW], F32, tag="bcat", name="bcat")
        nc.gpsimd.dma_start(bcat, b_d.ap())
        bo_sb = const.tile([1, OC], F32, tag="bo_sb", name="bo_sb")
        nc.gpsimd.dma_start(bo_sb, bo_d.ap())
        nc.sync.dma_start(x_sb[:, 0, 2:QH], xcat_d.ap()[:, 0, 2:QH])
        nc.scalar.dma_start(w_sb[:, 0, QH:KT], wcat_d.ap()[:, 0, QH:KT])
        nc.gpsimd.dma_start(x_sb[:, 0, QH:KT], xcat_d.ap()[:, 0, QH:KT])
        nc.scalar.dma_start(w_sb[:, 1:2], wcat_d.ap()[:, 1:2])   # q_var
        nc.sync.dma_start(x_sb[:, 1, 0:QH], xcat_d.ap()[:, 1, 0:QH])
        nc.gpsimd.dma_start(w_sb[:, 2:3], wcat_d.ap()[:, 2:3])   # v_var
        nc.scalar.dma_start(w_sb[:, 3:4], wcat_d.ap()[:, 3:4])   # k_mu
        nc.sync.dma_start(x_sb[:, 1, QH:KT], xcat_d.ap()[:, 1, QH:KT])
        nc.scalar.dma_start(w_sb[:, 4:5], wcat_d.ap()[:, 4:5])   # q_mu
        nc.gpsimd.dma_start(w_sb[:, 5:6], wcat_d.ap()[:, 5:6])   # v_mu
        # wo (1MB) is deferred: its dma_start is emitted after staging
        # so it doesn't compete with the projection streaming.

        # ---------------- constants -----------------------------------
        ident_b = const.tile([P, P], BF16, tag="ident_b", name="ident_b")
        make_identity(nc, ident_b)
        # sel[h]: 0.5 on head-h feature rows -> lhsT that sums r over a
        # head's t_s rows straight into the score PSUM
        sel = []
        for h in range(2):
            sh = const.tile([P, P], BF16, tag=f"sel{h}", name=f"sel{h}")
            nc.vector.memset(sh, 0.0)
            nc.vector.memset(sh[ts(h, Dh), :], 0.5)
            sel.append(sh)
        ones1 = const.tile([1, P], BF16, tag="ones1", name="ones1")
        nc.vector.memset(ones1, 1.0)

        B_ORDER = ["bk_var", "bq_var", "bv_var", "bk_mu", "bq_mu", "bv_mu"]
        nbcat = const.tile([CB, 3], F32, tag="nbcat", name="nbcat")
        nc.vector.tensor_scalar_mul(nbcat, bcat[:, 0:3], -1.0)
        bias = {n: bcat[:, i:i + 1] for i, n in enumerate(B_ORDER)}
        nbias = {n: nbcat[:, i:i + 1] for i, n in enumerate(B_ORDER[:3])}
        bo_bf = const.tile([1, OC], BF16, tag="bo_bf", name="bo_bf")
        nc.vector.tensor_copy(bo_bf, bo_sb)

        # warm the sigmoid table while DMAs run
        warm_sg = const.tile([1, 1], F32, tag="warm_sg", name="warm_sg")
        nc.scalar.activation(warm_sg, ident_b[0:1, 0:1], AF.Sigmoid)

        # ---------------- projections (feature-major [CB, L], bf16) ----
        def project(wi, si):
            ps = ps_proj.tile([P, L], F32, tag="proj", name="proj")
            for kt in range(KT):
                nc.tensor.matmul(ps, w_sb[:, wi, kt, :], x_sb[:, si, kt, :],
                                 start=(kt == 0), stop=(kt == KT - 1))
            return ps

        # --- kv projection: the ONLY sigmoid in the program, so the
        # table-aware scheduler cannot batch it with later work and the
        # Kv -> lnKv -> 1/Kv chain resolves during the projections.
        ps_kv = project(0, 0)
        sg_k = feat.tile([P, L], F32, tag="sg_k", name="sg_k")
        nc.scalar.activation(sg_k, ps_kv, AF.Sigmoid,
                             scale=-1.0, bias=nbias["bk_var"])
        nsp_k = feat.tile([P, L], F32, tag="nsp_k", name="nsp_k")
        nc.scalar.activation(nsp_k, sg_k, AF.Ln)          # -softplus
        t_lg = feat.tile([P, L], F32, tag="t_lg", name="t_lg")
        nc.scalar.activation(t_lg, nsp_k, AF.Ln, scale=-1.0)  # ln Kv
        t_iv = feat.tile([P, L], F32, tag="t_iv", name="t_iv")
        nc.scalar.activation(t_iv, t_lg, AF.Exp, scale=-1.0)  # 1/Kv

        # --- qv, vv: softplus = ln(1+exp) -- the dependency ladder
        # LN{nsp_k,t_lg} -> EXP{t_iv,e_q,e_v} -> LN{sp_q,vvT} forces the
        # scheduler to keep this order (loads hide under matmuls).
        ps_qv = project(1, 0)
        e_q = feat.tile([P, L], F32, tag="e_q", name="e_q")
        nc.scalar.activation(e_q, ps_qv, AF.Exp, bias=bias["bq_var"])
        w_q = feat.tile([P, L], F32, tag="w_q", name="w_q")
        nc.vector.tensor_scalar_add(w_q, e_q, 1.0)
        sp_q = feat.tile([P, L], F32, tag="sp_q", name="sp_q")
        nc.scalar.activation(sp_q, w_q, AF.Ln)            # Qv

        ps_vv = project(2, 0)
        e_v = feat.tile([P, L], F32, tag="e_v", name="e_v")
        nc.scalar.activation(e_v, ps_vv, AF.Exp, bias=bias["bv_var"])
        w_v = feat.tile([P, L], F32, tag="w_v", name="w_v")
        nc.vector.tensor_scalar_add(w_v, e_v, 1.0)

        vvT = feat.tile([P, L], BF16, tag="vvT", name="vvT")
        nc.scalar.activation(vvT, w_v, AF.Ln)             # Vv (bf16)

        # --- km projection + the whole r/fk feature chain ---------------
        ps_km = project(3, 1)
        t_km = feat.tile([P, L], F32, tag="t_km", name="t_km")
        nc.vector.tensor_scalar_add(t_km, ps_km, bias["bk_mu"])
        t_km2 = feat.tile([P, L], F32, tag="t_km2", name="t_km2")
        nc.vector.tensor_mul(t_km2, t_km, t_km)
        kmiv_f = feat.tile([P, L], F32, tag="kmiv_f", name="kmiv_f")
        nc.vector.tensor_mul(kmiv_f, t_km, t_iv)          # Km/Kv
        t_u = feat.tile([P, L], F32, tag="t_u", name="t_u")
        nc.vector.tensor_mul(t_u, t_km2, t_iv)            # Km^2/Kv
        t_s = feat.tile([P, L], F32, tag="t_s", name="t_s")
        nc.vector.tensor_add(t_s, t_u, t_lg)              # Km^2/Kv + lnKv
        ts_hi = feat.tile([P, L], BF16, tag="ts_hi", name="ts_hi")
        nc.vector.tensor_copy(ts_hi, t_s)
        ts_lo = feat.tile([P, L], BF16, tag="ts_lo", name="ts_lo")
        nc.vector.tensor_sub(ts_lo, t_s, ts_hi)
        fkcat = []
        for h in range(2):
            hs = ts(h, Dh)
            fk = feat.tile([P, L], BF16, tag=f"fk{h}", name=f"fk{h}")
            nc.vector.tensor_scalar_mul(fk[0:Dh, :], t_iv[hs, :], 0.5)
            nc.vector.tensor_copy(fk[Dh:P, :], kmiv_f[hs, :])
            fkcat.append(fk)

        ps_qm = project(4, 1)
        t_qm = feat.tile([P, L], F32, tag="t_qm", name="t_qm")
        nc.vector.tensor_scalar_add(t_qm, ps_qm, bias["bq_mu"])
        t_qm2 = feat.tile([P, L], F32, tag="t_qm2", name="t_qm2")
        nc.vector.tensor_mul(t_qm2, t_qm, t_qm)

        # --- scores + softmax for all 4 (h,t) tiles (before vm-proj) ----
        cc_in = dram.tile([NCORES * P, P], BF16, tag="cc_in", name="cc_in")
        pv_mu = ps_pv.tile([P, L], F32, tag="pv_mu", name="pv_mu")
        pv_var = ps_pv.tile([P, L], F32, tag="pv_var", name="pv_var")
        o_mu = attnp.tile([P, L], BF16, tag="o_mu", name="o_mu", bufs=1)
        o_var = attnp.tile([P, L], BF16, tag="o_var", name="o_var", bufs=1)

        a_bfs = {}
        fqcat = []
        for h in range(2):
            hs = ts(h, Dh)
            fq = feat.tile([P, L], BF16, tag=f"fq{h}", name=f"fq{h}")
            nc.vector.tensor_add(fq[0:Dh, :], t_qm2[hs, :], sp_q[hs, :])
            nc.vector.tensor_scalar_mul(fq[Dh:P, :], t_qm[hs, :], -1.0)
            fqcat.append(fq)
            for t in range(LT):
                ps_S = ps_s.tile([P, L], F32, tag="scores", name="scores")
                nc.tensor.matmul(ps_S, fq[:, ts(t, P)], fkcat[h],
                                 start=True, stop=False)
                nc.tensor.matmul(ps_S, sel[h], ts_hi,
                                 start=False, stop=False)
                nc.tensor.matmul(ps_S, sel[h], ts_lo,
                                 start=False, stop=True)
                pexp = attnp.tile([P, L], BF16, tag="pexp", name="pexp")
                den = attnp.tile([P, 1], F32, tag="den", name="den")
                nc.scalar.activation(pexp, ps_S, AF.Exp, bias=0.0,
                                     scale=-0.125, accum_out=den)
                invd = attnp.tile([P, 1], F32, tag="invd", name="invd")
                nc.vector.reciprocal(invd, den)
                a_bf = attnp.tile([P, L], BF16, tag=f"a_bf_{h}_{t}",
                                  name=f"a_bf_{h}_{t}", bufs=1)
                nc.vector.tensor_scalar_mul(a_bf, pexp, invd)
                a_bfs[(h, t)] = a_bf

        # --- vm projection + V transposes (softmax runs concurrently) ---
        ps_vm = project(5, 1)
        vmT = feat.tile([P, L], BF16, tag="vmT", name="vmT")
        nc.vector.tensor_scalar_add(vmT, ps_vm, bias["bv_mu"])
        v_l = {}
        for nm, src in (("vm", vmT), ("vv", vvT)):
            for lk in range(LT):
                pt = ps_tr.tile([P, P], BF16, tag="tr", name="trb")
                nc.tensor.transpose(pt, src[:, ts(lk, P)], ident_b)
                dst = feat.tile([P, P], BF16, tag=f"vl_{nm}_{lk}",
                                name=f"vl_{nm}_{lk}")
                nc.vector.tensor_copy(dst, pt)
                v_l[(nm, lk)] = dst

        # --- attention transposes, squares, PV (per head) ---------------
        attnT = {}
        a2T = {}
        for h in range(2):
            hs = ts(h, Dh)
            for t in range(LT):
                for lk in range(LT):
                    if (h, lk) not in attnT:
                        attnT[(h, lk)] = feat.tile(
                            [P, L], BF16, tag=f"attnT_{h}_{lk}",
                            name=f"attnT_{h}_{lk}")
                        a2T[(h, lk)] = feat.tile(
                            [P, L], BF16, tag=f"a2T_{h}_{lk}",
                            name=f"a2T_{h}_{lk}")
                    pt = ps_tr.tile([P, P], BF16, tag="tr", name="trb")
                    nc.tensor.transpose(pt, a_bfs[(h, t)][:, ts(lk, P)],
                                        ident_b)
                    nc.vector.tensor_copy(attnT[(h, lk)][:, ts(t, P)], pt)
            for lk in range(LT):
                nc.vector.tensor_mul(a2T[(h, lk)], attnT[(h, lk)],
                                     attnT[(h, lk)])
            for vkey, att, pv, o in (("vm", attnT, pv_mu, o_mu),
                                     ("vv", a2T, pv_var, o_var)):
                for lk in range(LT):
                    nc.tensor.matmul(pv[hs, :], v_l[(vkey, lk)][:, hs],
                                     att[(h, lk)],
                                     start=(lk == 0), stop=(lk == LT - 1),
                                     tile_position=(0, h * Dh))
                nc.scalar.copy(o[hs, :], pv[hs, :])

        # staging: chunk j rows (b*2+two)*128+p get O[p, 128*b + r]
        # (gpsimd + sync queues are idle here; scalar still runs copies)
        for row0, o, eng in ((0, o_mu, nc.gpsimd), (4 * P, o_var, nc.sync)):
            for b in range(2):
                r0 = row0 + b * 2 * P
                dst = cc_in[r0:r0 + 2 * P, :].rearrange(
                    "(two p) r -> p two r", two=2, p=P)
                src = o[:, ts(b, P)].unsqueeze(1)
                eng.dma_start(dst, src.broadcast_to([P, 2, P]))

        # deferred 1MB wo load: the scalar queue is busy with attention
        # copies until now, so these descriptors can't enter the DMA
        # engines early and steal bandwidth from the projection weights.
        # Lands during the collective; out-proj needs it ~30us later.
        nc.scalar.dma_start(wo_sb[:, 0:KT // 2], wocat_d.ap()[:, 0:KT // 2])
        nc.sync.dma_start(wo_sb[:, KT // 2:KT], wocat_d.ap()[:, KT // 2:KT])

        cc_out = dram.tile([NCORES * P, P], BF16, tag="cc_out",
                           name="cc_out")
        nc.gpsimd.collective_compute(
            "AllToAll", ALU.bypass,
            replica_groups=[list(range(NCORES))],
            ins=[cc_in[:].opt()],
            outs=[cc_out[:].opt()],
        )

        # ---------------- output projection (128 rows x 512 cols) ------
        g = persist.tile([P, KT, P], BF16, tag="gall", name="gall")
        rr = cc_out.rearrange("(c p) r -> p c r", p=P)
        for ci, eng in enumerate((nc.sync, nc.scalar, nc.gpsimd)):
            lo = (0, 3, 6)[ci]
            hi = (3, 6, 8)[ci]
            eng.dma_start(g[:, lo:hi], rr[:, lo:hi])

        # reuses the projection PSUM ring (projections are long done)
        ps_o = ps_proj.tile([P, OC], F32, tag="proj", name="ps_o")
        for kt in range(KT):
            nc.tensor.matmul(ps_o, g[:, kt, :], wo_sb[:, kt, :],
                             start=(kt == 0), stop=False)
        # rank-1 bias add (exact here since biases are zero; bf16 generally)
        nc.tensor.matmul(ps_o, ones1, bo_bf, start=False, stop=True)

        res_a = stage.tile([P, OC], F32, tag="res_a", name="res_a")
        nc.vector.tensor_copy(res_a, ps_o)
        nc.gpsimd.dma_start(out_a_d.ap(), res_a)
        u = stage.tile([P, OC], F32, tag="u", name="u")
        nc.scalar.activation(u, ps_o, AF.Exp)
        w1 = stage.tile([P, OC], F32, tag="w1", name="w1")
        nc.vector.tensor_scalar_add(w1, u, 1.0)
        res_b = stage.tile([P, OC], F32, tag="res_b", name="res_b")
        nc.scalar.activation(res_b, w1, AF.Ln)
        nc.sync.dma_start(out_b_d.ap(), res_b)


def shard_inputs(inputs):
    """Full inputs -> per-core in_maps (host-side numpy prep only)."""
    f32 = np.float32
    bf16 = mybir.dt.np(BF16)

    def to_pe_tiles(a):      # [1024, n] -> [128, 8, n]
        n = a.shape[1]
        return np.ascontiguousarray(
            a.reshape(KT, P, n).transpose(1, 0, 2))

    xcat = np.empty((P, 2, KT, L), dtype=bf16)
    for si, nm in enumerate(("var", "mu")):
        xt = np.asarray(inputs[nm]).reshape(L, D).astype(f32).T  # [D, L]
        xcat[:, si] = to_pe_tiles(xt.astype(bf16))

    W_ORDER = ["wk_var", "wq_var", "wv_var", "wk_mu", "wq_mu", "wv_mu"]
    B_ORDER = ["bk_var", "bq_var", "bv_var", "bk_mu", "bq_mu", "bv_mu"]
    in_maps = []
    for c in range(NCORES):
        cols = slice(c * CB, (c + 1) * CB)
        wcat = np.empty((P, NW, KT, CB), dtype=bf16)
        for wi, nm in enumerate(W_ORDER):
            w = np.asarray(inputs[nm])[:, cols].astype(f32).astype(bf16)
            wcat[:, wi] = to_pe_tiles(w)
        # output-role: half (mu for c<4), col block cblk
        half, cblk = c // 4, c % 2
        if half == 0:
            wo, bo = inputs["wo_mu"], inputs["bo_mu"]
        else:
            wo, bo = inputs["wo_var"], inputs["bo_var"]
        ocols = slice(cblk * OC, (cblk + 1) * OC)
        wocat = to_pe_tiles(np.asarray(wo)[:, ocols].astype(f32).astype(bf16))
        bcols = [np.asarray(inputs[n])[cols].astype(f32) for n in B_ORDER]
        biases = np.ascontiguousarray(np.stack(bcols, axis=1))
        b_out = np.ascontiguousarray(
            np.asarray(bo)[ocols].astype(f32)[None, :])
        in_maps.append({"xcat": xcat, "wcat": wcat, "wocat": wocat,
                        "biases": biases, "b_out": b_out})
    return in_maps


def kernel(**inputs):
    global LAST_RESULT
    if "prog" not in _prog_cache:
        _prog_cache["prog"] = build_program()
    nc = _prog_cache["prog"]
    in_maps = shard_inputs(inputs)
    res = run_bass_kernel_spmd(nc, in_maps, core_ids=list(range(NCORES)),
                               trace=TRACE, **TRACE_KWARGS)
    LAST_RESULT = res
    mu_out = np.empty((L, D), dtype=np.float32)
    var_out = np.empty((L, D), dtype=np.float32)
    for c in range(NCORES):
        half, rblk, cblk = c // 4, (c % 4) // 2, c % 2
        rows = slice(rblk * P, (rblk + 1) * P)
        cols = slice(cblk * OC, (cblk + 1) * OC)
        if half == 0:
            mu_out[rows, cols] = res.results[c]["out_a"]
        else:
            var_out[rows, cols] = res.results[c]["out_b"]
    return (np.ascontiguousarray(mu_out.reshape(B, L, D)),
            np.ascontiguousarray(var_out.reshape(B, L, D)))


# revision 41
# speedup vs baseline: 2.1847x; 2.1847x over previous
"""Distributional (Gaussian-KL) attention on 8 TRN2 NeuronCores.

Math: for each head, the KL-based score decomposes as
    kl[q,k] = sum_d (Qm-Km)^2/(2Kv) + 0.5*(Qv/Kv - log(Qv/Kv) - 1)
            = Fq[q] . Fk[k] + r[k] + c[q]
with  Fq = [Qm^2+Qv ; -Qm],  Fk = [1/(2Kv) ; Km/Kv],
      r  = 0.5*sum_d (Km^2/Kv + log Kv),
and c[q] only shifts softmax logits per-row (drops out of softmax over k).

Sharding: head-parallel front (core c owns heads {2c,2c+1} == feature
columns [128c,128c+128) of every Q/K/V projection).  Output stage is
ROW-split: core j owns out rows [128*rblk, +128) x cols [512*cblk, +512)
of ONE half (j<4: mu, j>=4: var), rblk=(j%4)//2, cblk=j%2.  One 256KB
AllToAll (chunk j = our O_half[:, rows_j], 32KB) delivers each core the
full 1024-feature contraction for its rows; out-proj runs at full PE
utilization (lhsT = [128 feats, 128 rows], rhs = 512-col wo block).

Scores per (head, q-tile): one 128-contraction matmul of packed
features fqcat=[Fq1;-Qm] x fkcat=[1/2Kv;Km/Kv], plus one matmul adding
r via an exact bf16 hi+lo split of t_s = Km^2/Kv + lnKv against an
all-0.5 lhsT (hi rows sum exactly in fp32 PSUM, lo carries residuals).

Engine budget: input DMAs fan out over 5 queues (tensor/scalar/gpsimd/
vector/sync) so projections never starve; the 1MB wo block + biases
load late (wo is only needed ~40us in).  ACT does tables + all
PSUM->SBUF copies + softmax normalize (copy/identity are in every ACT
table set, so no table thrash); DVE keeps the feature chain,
reciprocals, and squares.  Output bias is a rank-1 matmul.
"""

import numpy as np

import concourse.bass as bass
import concourse.mybir as mybir
import concourse.tile as tile
from concourse import bacc
from concourse.masks import make_identity
from concourse.bass_utils import run_bass_kernel_spmd

F32 = mybir.dt.float32
BF16 = mybir.dt.bfloat16
AF = mybir.ActivationFunctionType
ALU = mybir.AluOpType

H, B, L, D = 16, 1, 256, 1024
Dh = D // H          # 64
NCORES = 8
CB = D // NCORES     # 128 feature columns per core (2 heads)
P = 128
LT = L // P          # 2 row tiles of the sequence
KT = D // P          # 8 contraction tiles
NW = 6               # projection weight slots: kv qv vv km qm vm
OC = 512             # out-proj column block per core

TRACE = False
TRACE_KWARGS = {}
LAST_RESULT = None

_prog_cache = {}


def ts(i, size):
    return slice(i * size, (i + 1) * size)


def build_program():
    nc = bacc.Bacc("TRN2", target_bir_lowering=False, debug=False,
                   num_devices=NCORES)

    # xcat[p, s, kt, l] = x_s[l, kt*128+p]; s: 0=var, 1=mu
    xcat_d = nc.dram_tensor("xcat", [P, 2, KT, L], BF16, kind="ExternalInput")
    # wcat[p, w, kt, m] = w[kt*128+p, 128c+m]; slots kv qv vv km qm vm
    wcat_d = nc.dram_tensor("wcat", [P, NW, KT, CB], BF16,
                            kind="ExternalInput")
    # wocat[p, kt, m] = wo_half[kt*128+p, 512*cblk+m]
    wocat_d = nc.dram_tensor("wocat", [P, KT, OC], BF16,
                             kind="ExternalInput")
    b_d = nc.dram_tensor("biases", [CB, NW], F32, kind="ExternalInput")
    bo_d = nc.dram_tensor("b_out", [1, OC], F32, kind="ExternalInput")
    # out_a: identity epilogue (mu), out_b: softplus epilogue (var)
    out_a_d = nc.dram_tensor("out_a", [P, OC], F32, kind="ExternalOutput")
    out_b_d = nc.dram_tensor("out_b", [P, OC], F32, kind="ExternalOutput")

    with tile.TileContext(nc) as tc:
        _build(nc, tc, xcat_d, wcat_d, wocat_d, b_d, bo_d, out_a_d, out_b_d)
    nc.compile()
    return nc


def _build(nc, tc, xcat_d, wcat_d, wocat_d, b_d, bo_d, out_a_d, out_b_d):
    from contextlib import ExitStack
    ctx = ExitStack()
    with ctx:
        const = ctx.enter_context(tc.tile_pool(name="const", bufs=1))
        persist = ctx.enter_context(tc.tile_pool(name="persist", bufs=1))
        stage = ctx.enter_context(tc.tile_pool(name="stage", bufs=1))
        feat = ctx.enter_context(tc.tile_pool(name="feat", bufs=1))
        attnp = ctx.enter_context(tc.tile_pool(name="attnp", bufs=2))
        ps_proj = ctx.enter_context(
            tc.tile_pool(name="ps_proj", bufs=2, space="PSUM"))
        ps_tr = ctx.enter_context(
            tc.tile_pool(name="ps_tr", bufs=2, space="PSUM"))
        ps_s = ctx.enter_context(
            tc.tile_pool(name="ps_s", bufs=2, space="PSUM"))
        ps_pv = ctx.enter_context(
            tc.tile_pool(name="ps_pv", bufs=1, space="PSUM"))
        dram = ctx.enter_context(tc.tile_pool(name="dram", bufs=1,
                                              space="DRAM"))

        # ------- input DMAs: 5 issue queues, critical operands first ----
        x_sb = persist.tile([P, 2, KT, L], BF16, tag="x_sb", name="x_sb")
        w_sb = persist.tile([P, NW, KT, CB], BF16, tag="w_sb", name="w_sb")
        wo_sb = persist.tile([P, KT, OC], BF16, tag="wo_sb", name="wo_sb")
        QH = KT // 2
        # w slots in consumption order: kv qv vv km qm vm.  Early window
        # runs ~55GB/s per queue, so x rides two queues and no queue
        # carries more than ~384KB of the first-needed bytes.
        nc.sync.dma_start(x_sb[:, 0, 0:2], xcat_d.ap()[:, 0, 0:2])
        nc.scalar.dma_start(w_sb[:, 0, 0:QH], wcat_d.ap()[:, 0, 0:QH])
        bcat = const.tile([CB, NW], F32, tag="bcat", name="bcat")
        nc.gpsimd.dma_start(bcat, b_d.ap())
        bo_sb = const.tile([1, OC], F32, tag="bo_sb", name="bo_sb")
        nc.gpsimd.dma_start(bo_sb, bo_d.ap())
        nc.sync.dma_start(x_sb[:, 0, 2:QH], xcat_d.ap()[:, 0, 2:QH])
        nc.scalar.dma_start(w_sb[:, 0, QH:KT], wcat_d.ap()[:, 0, QH:KT])
        nc.gpsimd.dma_start(x_sb[:, 0, QH:KT], xcat_d.ap()[:, 0, QH:KT])
        nc.scalar.dma_start(w_sb[:, 1:2], wcat_d.ap()[:, 1:2])   # q_var
        nc.sync.dma_start(x_sb[:, 1, 0:QH], xcat_d.ap()[:, 1, 0:QH])
        nc.gpsimd.dma_start(w_sb[:, 2:3], wcat_d.ap()[:, 2:3])   # v_var
        nc.scalar.dma_start(w_sb[:, 3:4], wcat_d.ap()[:, 3:4])   # k_mu
        nc.sync.dma_start(x_sb[:, 1, QH:KT], xcat_d.ap()[:, 1, QH:KT])
        nc.scalar.dma_start(w_sb[:, 4:5], wcat_d.ap()[:, 4:5])   # q_mu
        nc.gpsimd.dma_start(w_sb[:, 5:6], wcat_d.ap()[:, 5:6])   # v_mu
        # wo (1MB) is deferred: its dma_start is emitted after staging
        # so it doesn't compete with the projection streaming.

        # ---------------- constants -----------------------------------
        ident_b = const.tile([P, P], BF16, tag="ident_b", name="ident_b")
        make_identity(nc, ident_b)
        # sel[h]: 0.5 on head-h feature rows -> lhsT that sums r over a
        # head's t_s rows straight into the score PSUM
        sel = []
        for h in range(2):
            sh = const.tile([P, P], BF16, tag=f"sel{h}", name=f"sel{h}")
            nc.vector.memset(sh, 0.0)
            nc.vector.memset(sh[ts(h, Dh), :], 0.5)
            sel.append(sh)
        ones1 = const.tile([1, P], BF16, tag="ones1", name="ones1")
        nc.vector.memset(ones1, 1.0)

        B_ORDER = ["bk_var", "bq_var", "bv_var", "bk_mu", "bq_mu", "bv_mu"]
        bias = {n: bcat[:, i:i + 1] for i, n in enumerate(B_ORDER)}
        bo_bf = const.tile([1, OC], BF16, tag="bo_bf", name="bo_bf")
        nc.vector.tensor_copy(bo_bf, bo_sb)

        # warm the exp table while DMAs run (first table used)
        warm_ex = const.tile([1, 1], F32, tag="warm_ex", name="warm_ex")
        nc.scalar.activation(warm_ex, ident_b[0:1, 0:1], AF.Exp)

        # ---------------- projections (feature-major [CB, L], bf16) ----
        def project(wi, si):
            ps = ps_proj.tile([P, L], F32, tag="proj", name="proj")
            for kt in range(KT):
                nc.tensor.matmul(ps, w_sb[:, wi, kt, :], x_sb[:, si, kt, :],
                                 start=(kt == 0), stop=(kt == KT - 1))
            return ps

        # --- kv projection: the ONLY sigmoid in the program, so the
        # table-aware scheduler cannot batch it with later work and the
        # Kv -> lnKv -> 1/Kv chain resolves during the projections.
        # All softplus via ln(1+exp): one EXP batch then one LN batch
        # (exp table warm from t=0; no sigmoid for the scheduler to
        # re-batch).
        ps_kv = project(0, 0)
        e_k = feat.tile([P, L], F32, tag="e_k", name="e_k")
        nc.scalar.activation(e_k, ps_kv, AF.Exp, bias=bias["bk_var"])
        w_k = feat.tile([P, L], F32, tag="w_k", name="w_k")
        nc.vector.tensor_scalar_add(w_k, e_k, 1.0)

        ps_qv = project(1, 0)
        e_q = feat.tile([P, L], F32, tag="e_q", name="e_q")
        nc.scalar.activation(e_q, ps_qv, AF.Exp, bias=bias["bq_var"])
        w_q = feat.tile([P, L], F32, tag="w_q", name="w_q")
        nc.vector.tensor_scalar_add(w_q, e_q, 1.0)

        ps_vv = project(2, 0)
        e_v = feat.tile([P, L], F32, tag="e_v", name="e_v")
        nc.scalar.activation(e_v, ps_vv, AF.Exp, bias=bias["bv_var"])
        w_v = feat.tile([P, L], F32, tag="w_v", name="w_v")
        nc.vector.tensor_scalar_add(w_v, e_v, 1.0)

        # LN batch: Kv, lnKv, Qv, Vv; 1/Kv on DVE off the table path
        t_kv = feat.tile([P, L], F32, tag="t_kv", name="t_kv")
        nc.scalar.activation(t_kv, w_k, AF.Ln)            # Kv
        t_lg = feat.tile([P, L], F32, tag="t_lg", name="t_lg")
        nc.scalar.activation(t_lg, t_kv, AF.Ln)           # ln Kv
        t_iv = feat.tile([P, L], F32, tag="t_iv", name="t_iv")
        nc.vector.reciprocal(t_iv, t_kv)                  # 1/Kv
        sp_q = feat.tile([P, L], F32, tag="sp_q", name="sp_q")
        nc.scalar.activation(sp_q, w_q, AF.Ln)            # Qv
        vvT = feat.tile([P, L], BF16, tag="vvT", name="vvT")
        nc.scalar.activation(vvT, w_v, AF.Ln)             # Vv (bf16)

        # --- km projection + the whole r/fk feature chain ---------------
        ps_km = project(3, 1)
        t_km = feat.tile([P, L], F32, tag="t_km", name="t_km")
        nc.vector.tensor_scalar_add(t_km, ps_km, bias["bk_mu"])
        t_km2 = feat.tile([P, L], F32, tag="t_km2", name="t_km2")
        nc.vector.tensor_mul(t_km2, t_km, t_km)
        kmiv_f = feat.tile([P, L], F32, tag="kmiv_f", name="kmiv_f")
        nc.vector.tensor_mul(kmiv_f, t_km, t_iv)          # Km/Kv
        t_u = feat.tile([P, L], F32, tag="t_u", name="t_u")
        nc.vector.tensor_mul(t_u, t_km2, t_iv)            # Km^2/Kv
        t_s = feat.tile([P, L], F32, tag="t_s", name="t_s")
        nc.vector.tensor_add(t_s, t_u, t_lg)              # Km^2/Kv + lnKv
        ts_hi = feat.tile([P, L], BF16, tag="ts_hi", name="ts_hi")
        nc.vector.tensor_copy(ts_hi, t_s)
        ts_lo = feat.tile([P, L], BF16, tag="ts_lo", name="ts_lo")
        nc.vector.tensor_sub(ts_lo, t_s, ts_hi)
        fkcat = []
        for h in range(2):
            hs = ts(h, Dh)
            fk = feat.tile([P, L], BF16, tag=f"fk{h}", name=f"fk{h}")
            nc.vector.tensor_scalar_mul(fk[0:Dh, :], t_iv[hs, :], 0.5)
            nc.vector.tensor_copy(fk[Dh:P, :], kmiv_f[hs, :])
            fkcat.append(fk)

        ps_qm = project(4, 1)
        t_qm = feat.tile([P, L], F32, tag="t_qm", name="t_qm")
        nc.vector.tensor_scalar_add(t_qm, ps_qm, bias["bq_mu"])
        t_qm2 = feat.tile([P, L], F32, tag="t_qm2", name="t_qm2")
        nc.vector.tensor_mul(t_qm2, t_qm, t_qm)

        # --- scores + softmax for all 4 (h,t) tiles (before vm-proj) ----
        cc_in = dram.tile([NCORES * P, P], BF16, tag="cc_in", name="cc_in")
        pv_mu = ps_pv.tile([P, L], F32, tag="pv_mu", name="pv_mu")
        pv_var = ps_pv.tile([P, L], F32, tag="pv_var", name="pv_var")
        o_mu = attnp.tile([P, L], BF16, tag="o_mu", name="o_mu", bufs=1)
        o_var = attnp.tile([P, L], BF16, tag="o_var", name="o_var", bufs=1)

        a_bfs = {}
        fqcat = []
        for h in range(2):
            hs = ts(h, Dh)
            fq = feat.tile([P, L], BF16, tag=f"fq{h}", name=f"fq{h}")
            nc.vector.tensor_add(fq[0:Dh, :], t_qm2[hs, :], sp_q[hs, :])
            nc.vector.tensor_scalar_mul(fq[Dh:P, :], t_qm[hs, :], -1.0)
            fqcat.append(fq)
            for t in range(LT):
                ps_S = ps_s.tile([P, L], F32, tag="scores", name="scores")
                nc.tensor.matmul(ps_S, fq[:, ts(t, P)], fkcat[h],
                                 start=True, stop=False)
                nc.tensor.matmul(ps_S, sel[h], ts_hi,
                                 start=False, stop=False)
                nc.tensor.matmul(ps_S, sel[h], ts_lo,
                                 start=False, stop=True)
                pexp = attnp.tile([P, L], BF16, tag="pexp", name="pexp")
                den = attnp.tile([P, 1], F32, tag="den", name="den")
                nc.scalar.activation(pexp, ps_S, AF.Exp, bias=0.0,
                                     scale=-0.125, accum_out=den)
                invd = attnp.tile([P, 1], F32, tag="invd", name="invd")
                nc.vector.reciprocal(invd, den)
                a_bf = attnp.tile([P, L], BF16, tag=f"a_bf_{h}_{t}",
                                  name=f"a_bf_{h}_{t}", bufs=1)
                nc.vector.tensor_scalar_mul(a_bf, pexp, invd)
                a_bfs[(h, t)] = a_bf

        # --- vm projection + V transposes (softmax runs concurrently) ---
        ps_vm = project(5, 1)
        vmT = feat.tile([P, L], BF16, tag="vmT", name="vmT")
        nc.vector.tensor_scalar_add(vmT, ps_vm, bias["bv_mu"])
        v_l = {}
        for nm, src in (("vm", vmT), ("vv", vvT)):
            for lk in range(LT):
                pt = ps_tr.tile([P, P], BF16, tag="tr", name="trb")
                nc.tensor.transpose(pt, src[:, ts(lk, P)], ident_b)
                dst = feat.tile([P, P], BF16, tag=f"vl_{nm}_{lk}",
                                name=f"vl_{nm}_{lk}")
                nc.vector.tensor_copy(dst, pt)
                v_l[(nm, lk)] = dst

        # --- attention transposes, squares, PV (per head) ---------------
        attnT = {}
        a2T = {}
        for h in range(2):
            hs = ts(h, Dh)
            for t in range(LT):
                for lk in range(LT):
                    if (h, lk) not in attnT:
                        attnT[(h, lk)] = feat.tile(
                            [P, L], BF16, tag=f"attnT_{h}_{lk}",
                            name=f"attnT_{h}_{lk}")
                        a2T[(h, lk)] = feat.tile(
                            [P, L], BF16, tag=f"a2T_{h}_{lk}",
                            name=f"a2T_{h}_{lk}")
                    pt = ps_tr.tile([P, P], BF16, tag="tr", name="trb")
                    nc.tensor.transpose(pt, a_bfs[(h, t)][:, ts(lk, P)],
                                        ident_b)
                    nc.vector.tensor_copy(attnT[(h, lk)][:, ts(t, P)], pt)
            for lk in range(LT):
                nc.vector.tensor_mul(a2T[(h, lk)], attnT[(h, lk)],
                                     attnT[(h, lk)])
            for vkey, att, pv, o in (("vm", attnT, pv_mu, o_mu),
                                     ("vv", a2T, pv_var, o_var)):
                for lk in range(LT):
                    nc.tensor.matmul(pv[hs, :], v_l[(vkey, lk)][:, hs],
                                     att[(h, lk)],
                                     start=(lk == 0), stop=(lk == LT - 1),
                                     tile_position=(0, h * Dh))
                nc.scalar.copy(o[hs, :], pv[hs, :])

        # staging: chunk j rows (b*2+two)*128+p get O[p, 128*b + r]
        # (gpsimd + sync queues are idle here; scalar still runs copies)
        for row0, o, eng in ((0, o_mu, nc.gpsimd), (4 * P, o_var, nc.sync)):
            for b in range(2):
                r0 = row0 + b * 2 * P
                dst = cc_in[r0:r0 + 2 * P, :].rearrange(
                    "(two p) r -> p two r", two=2, p=P)
                src = o[:, ts(b, P)].unsqueeze(1)
                eng.dma_start(dst, src.broadcast_to([P, 2, P]))

        # deferred 1MB wo load: the scalar queue is busy with attention
        # copies until now, so these descriptors can't enter the DMA
        # engines early and steal bandwidth from the projection weights.
        # Lands during the collective; out-proj needs it ~30us later.
        nc.scalar.dma_start(wo_sb[:, 0:KT // 2], wocat_d.ap()[:, 0:KT // 2])
        nc.sync.dma_start(wo_sb[:, KT // 2:KT], wocat_d.ap()[:, KT // 2:KT])

        cc_out = dram.tile([NCORES * P, P], BF16, tag="cc_out",
                           name="cc_out")
        nc.gpsimd.collective_compute(
            "AllToAll", ALU.bypass,
            replica_groups=[list(range(NCORES))],
            ins=[cc_in[:].opt()],
            outs=[cc_out[:].opt()],
        )

        # ---------------- output projection (128 rows x 512 cols) ------
        g = persist.tile([P, KT, P], BF16, tag="gall", name="gall")
        rr = cc_out.rearrange("(c p) r -> p c r", p=P)
        for ci, eng in enumerate((nc.sync, nc.scalar, nc.gpsimd)):
            lo = (0, 3, 6)[ci]
            hi = (3, 6, 8)[ci]
            eng.dma_start(g[:, lo:hi], rr[:, lo:hi])

        # reuses the projection PSUM ring (projections are long done)
        ps_o = ps_proj.tile([P, OC], F32, tag="proj", name="ps_o")
        for kt in range(KT):
            nc.tensor.matmul(ps_o, g[:, kt, :], wo_sb[:, kt, :],
                             start=(kt == 0), stop=False)
        # rank-1 bias add (exact here since biases are zero; bf16 generally)
        nc.tensor.matmul(ps_o, ones1, bo_bf, start=False, stop=True)

        res_a = stage.tile([P, OC], F32, tag="res_a", name="res_a")
        nc.vector.tensor_copy(res_a, ps_o)
        nc.gpsimd.dma_start(out_a_d.ap(), res_a)
        u = stage.tile([P, OC], F32, tag="u", name="u")
        nc.scalar.activation(u, ps_o, AF.Exp)
        w1 = stage.tile([P, OC], F32, tag="w1", name="w1")
        nc.vector.tensor_scalar_add(w1, u, 1.0)
        res_b = stage.tile([P, OC], F32, tag="res_b", name="res_b")
        nc.scalar.activation(res_b, w1, AF.Ln)
        nc.sync.dma_start(out_b_d.ap(), res_b)


def shard_inputs(inputs):
    """Full inputs -> per-core in_maps (host-side numpy prep only)."""
    f32 = np.float32
    bf16 = mybir.dt.np(BF16)

    def to_pe_tiles(a):      # [1024, n] -> [128, 8, n]
        n = a.shape[1]
        return np.ascontiguousarray(
            a.reshape(KT, P, n).transpose(1, 0, 2))

    xcat = np.empty((P, 2, KT, L), dtype=bf16)
    for si, nm in enumerate(("var", "mu")):
        xt = np.asarray(inputs[nm]).reshape(L, D).astype(f32).T  # [D, L]
        xcat[:, si] = to_pe_tiles(xt.astype(bf16))

    W_ORDER = ["wk_var", "wq_var", "wv_var", "wk_mu", "wq_mu", "wv_mu"]
    B_ORDER = ["bk_var", "bq_var", "bv_var", "bk_mu", "bq_mu", "bv_mu"]
    in_maps = []
    for c in range(NCORES):
        cols = slice(c * CB, (c + 1) * CB)
        wcat = np.empty((P, NW, KT, CB), dtype=bf16)
        for wi, nm in enumerate(W_ORDER):
            w = np.asarray(inputs[nm])[:, cols].astype(f32).astype(bf16)
            wcat[:, wi] = to_pe_tiles(w)
        # output-role: half (mu for c<4), col block cblk
        half, cblk = c // 4, c % 2
        if half == 0:
            wo, bo = inputs["wo_mu"], inputs["bo_mu"]
        else:
            wo, bo = inputs["wo_var"], inputs["bo_var"]
        ocols = slice(cblk * OC, (cblk + 1) * OC)
        wocat = to_pe_tiles(np.asarray(wo)[:, ocols].astype(f32).astype(bf16))
        bcols = [np.asarray(inputs[n])[cols].astype(f32) for n in B_ORDER]
        biases = np.ascontiguousarray(np.stack(bcols, axis=1))
        b_out = np.ascontiguousarray(
            np.asarray(bo)[ocols].astype(f32)[None, :])
        in_maps.append({"xcat": xcat, "wcat": wcat, "wocat": wocat,
                        "biases": biases, "b_out": b_out})
    return in_maps


def kernel(**inputs):
    global LAST_RESULT
    if "prog" not in _prog_cache:
        _prog_cache["prog"] = build_program()
    nc = _prog_cache["prog"]
    in_maps = shard_inputs(inputs)
    res = run_bass_kernel_spmd(nc, in_maps, core_ids=list(range(NCORES)),
                               trace=TRACE, **TRACE_KWARGS)
    LAST_RESULT = res
    mu_out = np.empty((L, D), dtype=np.float32)
    var_out = np.empty((L, D), dtype=np.float32)
    for c in range(NCORES):
        half, rblk, cblk = c // 4, (c % 4) // 2, c % 2
        rows = slice(rblk * P, (rblk + 1) * P)
        cols = slice(cblk * OC, (cblk + 1) * OC)
        if half == 0:
            mu_out[rows, cols] = res.results[c]["out_a"]
        else:
            var_out[rows, cols] = res.results[c]["out_b"]
    return (np.ascontiguousarray(mu_out.reshape(B, L, D)),
            np.ascontiguousarray(var_out.reshape(B, L, D)))
